# revision 1
# baseline (speedup 1.0000x reference)
"""Causal self-attention (B=4, T=2048, D=1024, H=16) on 8 TRN2 NeuronCores.

Sharding: tensor-parallel over heads. Each core owns 2 heads: it computes
Q/K/V projections for its head-slice of W_qkv (column-parallel), full causal
attention for those heads, and a partial output projection with its row-slice
of W_out (row-parallel). The host sums the 8 partials and adds b_out.

Per-core kernel layout (all matmuls bf16 with fp32 PSUM accumulation):
  - x is pre-transposed on the host to xT [D, B*T] so the projection
    contraction dim (D) lies on SBUF partitions.
  - Projections produce Q^T/K^T [n, t] directly (W chunks stationary,
    xT chunks moving); V is produced as V^T then PE-transposed to [t, dv].
  - Scores are computed transposed, S^T [keys, q], two heads packed into
    one PSUM tile via row-group tiling (contraction dim is 64 per head).
  - Softmax skips the max subtraction (scores are O(1) by construction:
    exp never overflows), so exp comes straight off PSUM via ScalarE.
  - The AV matmul's stationary operand is [V_h | ones*64] (128 cols), so
    partitions 64:128 of the O accumulator hold the softmax denominator
    replicated 64x - normalization is one reciprocal + one multiply.
  - Causality: key-chunk matmuls on the diagonal are narrowed to the
    valid query range; the 128x128 boundary subtile is masked with a
    triangular constant after exp.
  - The attention inner loop is ACT(exp)-bound, so independent PE work
    (next batch's projection + V transposes, previous block's output
    projection) is interleaved into it via filler generators to keep the
    TensorEngine dense (and the HAM clock-gate warm).
"""
import os
import numpy as np
import ml_dtypes
from contextlib import ExitStack

import concourse.bass as bass
import concourse.tile as tile
from concourse import bacc, mybir
from concourse.bass_utils import run_bass_kernel_spmd

# This kernel's only ACT functions are Exp and Ln, which share the
# natural_log_exp_and_others table set. By default the table-load pass maps
# Exp to the earlier exp_and_others set, thrashing two ~1.3us table reloads
# around every Ln pair. Narrow the pass's view so Exp resolves only to the
# shared set (list order is preserved, so emitted act_func_set_ids stay
# valid act_info.json indices).
_orig_gat = bacc.get_activation_tables


def _gat_ln_exp(arch):
    out = {}
    for name, funcs in _orig_gat(arch).items():
        if name != "natural_log_exp_and_others":
            funcs = funcs - {mybir.ActivationFunctionType.Exp}
        out[name] = funcs
    return out


bacc.get_activation_tables = _gat_ln_exp

N_CORES = 8
B, T, D = 4, 2048, 1024
H, DH = 16, 64
HPC = H // N_CORES          # heads per core = 2
BT = B * T                  # 8192
TPB = T // 512              # 4 token blocks per batch
NKC = T // 128              # 16 key chunks per batch
NQB = T // 512              # 4 query blocks per batch

F32 = mybir.dt.float32
BF16 = mybir.dt.bfloat16
EXPF = mybir.ActivationFunctionType.Exp

_CACHED_NC = None
LAST_RESULTS = None  # test harness reads exec_time from here


def _act_recip(nc, out, in_, scratch):
    """1/x on ScalarE as exp(-ln(x)). Ln and Exp share one ACT table set
    (natural_log_exp_and_others) so this costs no table reloads, unlike the
    Reciprocal table (2 reloads per use, ~2.6us). Roundtrip rel err ~1e-6
    for the softmax-denominator range, far below this kernel's bf16 noise
    floor; ~4.5x cheaper than the DVE iterative divide and runs on the
    less-loaded engine."""
    nc.scalar.activation(scratch, in_, mybir.ActivationFunctionType.Ln)
    nc.scalar.activation(out, scratch, EXPF, scale=-1.0)


def _build():
    nc = bacc.Bacc("TRN2", target_bir_lowering=False, debug=False,
                   num_devices=N_CORES)
    d_xT = nc.dram_tensor("xT", [D, BT], BF16, kind="ExternalInput").ap()
    d_wq = nc.dram_tensor("wq", [D, 128], BF16, kind="ExternalInput").ap()
    d_wk = nc.dram_tensor("wk", [D, 128], BF16, kind="ExternalInput").ap()
    d_wv = nc.dram_tensor("wv", [D, 128], BF16, kind="ExternalInput").ap()
    d_wo = nc.dram_tensor("wo", [128, D], BF16, kind="ExternalInput").ap()
    d_bias = nc.dram_tensor("bias", [128, 3], F32, kind="ExternalInput").ap()
    d_tri = nc.dram_tensor("tri", [128, 128], BF16, kind="ExternalInput").ap()
    d_ident = nc.dram_tensor("ident", [128, 128], BF16, kind="ExternalInput").ap()
    d_out = nc.dram_tensor("out", [BT, D], F32, kind="ExternalOutput").ap()

    with tile.TileContext(nc) as tc:
        with ExitStack() as ctx:
            consts = ctx.enter_context(tc.tile_pool(name="consts", bufs=1))
            big = ctx.enter_context(tc.tile_pool(name="big", bufs=1))
            vtpool = ctx.enter_context(tc.tile_pool(name="vt", bufs=2))
            xpool = ctx.enter_context(tc.tile_pool(name="xt", bufs=3))
            ppool = ctx.enter_context(tc.tile_pool(name="pt", bufs=6))
            opool = ctx.enter_context(tc.tile_pool(name="ot", bufs=12))
            rpool = ctx.enter_context(tc.tile_pool(name="rc", bufs=2))
            outp = ctx.enter_context(tc.tile_pool(name="outp", bufs=6))
            psA = ctx.enter_context(tc.tile_pool(name="psA", bufs=2, space="PSUM"))
            psO = ctx.enter_context(tc.tile_pool(name="psO", bufs=1, space="PSUM"))
            psM = ctx.enter_context(tc.tile_pool(name="psM", bufs=1, space="PSUM"))

            # ---- constants ----
            wq_sb = consts.tile([128, 1024], BF16, tag="wq")
            wk_sb = consts.tile([128, 1024], BF16, tag="wk")
            wv_sb = consts.tile([128, 1024], BF16, tag="wv")
            # (c p) n -> p (c n): k-chunk c of W lands at cols [c*128, c*128+128)
            for w_sb, d_w in ((wq_sb, d_wq), (wk_sb, d_wk), (wv_sb, d_wv)):
                nc.sync.dma_start(
                    w_sb[:].rearrange("p (c n) -> p c n", c=8),
                    d_w.rearrange("(c p) n -> p c n", p=128))
            wo_sb = consts.tile([128, 1024], BF16, tag="wo")
            nc.sync.dma_start(wo_sb[:], d_wo[:])
            bias_sb = consts.tile([128, 3], F32, tag="bias")
            nc.sync.dma_start(bias_sb[:], d_bias[:])
            tri_sb = consts.tile([128, 128], BF16, tag="tri")
            nc.sync.dma_start(tri_sb[:], d_tri[:])
            ident_sb = consts.tile([128, 128], BF16, tag="ident")
            nc.sync.dma_start(ident_sb[:], d_ident[:])

            # ---- persistent per-batch tensors ----
            qt = [big.tile([128, T], BF16, tag=f"qt{b}", name=f"qt{b}")
                  for b in range(B)]
            kt = [big.tile([128, T], BF16, tag=f"kt{b}", name=f"kt{b}")
                  for b in range(B)]
            # v_sb[b]: 16 key chunks x [V_h0 | ones | V_h1 | ones] (256 cols)
            v_sb = [big.tile([128, NKC * 256], BF16, tag=f"v{b}", name=f"v{b}")
                    for b in range(B)]
            for b in range(B):
                nc.gpsimd.memset(v_sb[b][:], 1.0)

            vt_tmp = [None] * B   # V^T staging per batch
            proj_prog = [0] * B   # completed t-blocks per batch
            proj_cap = [TPB] * B  # phase throttle: max t-blocks to emit

            def proj_gen(b):
                """Projection + V transpose of one t-block at a time, in
                small PE steps. attn(b, qb) only reads K/V key chunks up to
                t-block qb, so later t-blocks legally interleave INTO batch
                b's own attention - the filler that keeps the last batch's
                TensorEngine dense (and its HAM clock warm)."""
                vt_tmp[b] = vtpool.tile([128, T], BF16, tag="vt", name=f"vt{b}")
                for tbl in range(TPB):
                    x_t = xpool.tile([128, 8 * 512], BF16, tag="xt",
                                     name=f"x{b}_{tbl}")
                    for c in range(8):
                        nc.sync.dma_start(
                            x_t[:, bass.ts(c, 512)],
                            d_xT[c * 128: c * 128 + 128,
                                 bass.ts(b * TPB + tbl, 512)])
                    for pi, (w_sb, col) in enumerate(
                            ((wq_sb, 0), (wk_sb, 1), (wv_sb, 2))):
                        # alternate pools: double-buffers the accumulator
                        # across groups despite each pool having bufs=1
                        gpool, gtag = ((psM, "proj"), (psO, "proj2"))[
                            (tbl * 3 + pi) % 2]
                        ps = gpool.tile([128, 512], F32, tag=gtag, bufs=1,
                                        name=f"pj{b}_{tbl}_{pi}")
                        for c in range(8):
                            nc.tensor.matmul(
                                ps[:], w_sb[:, bass.ts(c, 128)],
                                x_t[:, bass.ts(c, 512)],
                                start=(c == 0), stop=(c == 7))
                            yield
                        dest = (qt[b], kt[b], vt_tmp[b])[pi]
                        nc.vector.tensor_scalar_add(
                            dest[:, bass.ts(tbl, 512)], ps[:],
                            bias_sb[:, col:col + 1])
                    # this t-block's V^T -> v_sb [t, (V|1|V|1)]: 4 PE
                    # transposes staged in the 1-bank psM pool (bitcast)
                    tp = psM.tile([128, 512], F32, tag="proj",
                                  name=f"tp{b}_{tbl}")
                    tpb = tp[:].bitcast(BF16)
                    for t4 in range(4):
                        tc16 = tbl * 4 + t4
                        nc.tensor.transpose(
                            tpb[:, t4 * 128: t4 * 128 + 128],
                            vt_tmp[b][:, bass.ts(tc16, 128)], ident_sb[:])
                    src = bass.AP(tpb.tensor, tpb.offset,
                                  [tpb.ap[0], [128, 4], [64, 2], [1, 64]])
                    dst0 = v_sb[b][:, tbl * 1024: tbl * 1024 + 1024]
                    dst = bass.AP(dst0.tensor, dst0.offset,
                                  [dst0.ap[0], [256, 4], [128, 2], [1, 64]])
                    nc.vector.tensor_copy(dst, src)
                    proj_prog[b] = tbl + 1
                    yield

            def outproj_gen(b, qb, o_sb):
                """out[q, n] = sum_dv O^T[dv, q] * W_out[dv, n], per q-chunk."""
                for qc in range(4):
                    op = psA.tile([128, 1024], F32, tag="sA",
                                  name=f"op{b}_{qb}_{qc}")
                    for n2 in range(2):
                        nc.tensor.matmul(
                            op[:, bass.ts(n2, 512)],
                            o_sb[:, bass.ts(qc, 128)],
                            wo_sb[:, bass.ts(n2, 512)],
                            start=True, stop=True)
                        yield
                    osb = outp.tile([128, 1024], F32, tag="outp",
                                    name=f"ob{b}_{qb}_{qc}")
                    nc.vector.tensor_copy(osb[:], op[:])
                    row = b * T + qb * 512 + qc * 128
                    nc.sync.dma_start(d_out[row:row + 128, :], osb[:])
                    yield

            fill_proj = []  # (batch, generator) projection fillers
            fill_op = []    # short out-projection generators


            def pull_from(lst, n):
                for _ in range(n):
                    while lst:
                        try:
                            next(lst[0])
                            break
                        except StopIteration:
                            lst.pop(0)
                    else:
                        break

            def pull_proj(n):
                got = 0
                for _ in range(n):
                    while fill_proj:
                        pb, g = fill_proj[0]
                        if proj_prog[pb] >= proj_cap[pb]:
                            return got  # head gen throttled for a later phase
                        try:
                            next(g)
                            got += 1
                            break
                        except StopIteration:
                            fill_proj.pop(0)
                    else:
                        break
                return got

            def force_proj(b, upto):
                """Emit batch b's projection through t-block `upto` NOW
                (earlier batches' leftovers drain first - they are older
                dependencies by construction)."""
                proj_cap[b] = max(proj_cap[b], upto)
                while proj_prog[b] < upto and fill_proj:
                    pb, g = fill_proj[0]
                    try:
                        next(g)
                    except StopIteration:
                        fill_proj.pop(0)

            def pull(n):
                # out-projections are short and slot-critical: keep them moving
                pull_from(fill_op, 1)
                pull_proj(n)

            def attn(b, qb):
                """Attention for query block qb of batch b."""
                # one accumulator for both heads (h0 cols 0:512, h1 cols
                # 512:1024): the denominator rows land contiguous at
                # [64:128, 0:1024], so one ln+exp pair normalizes both
                # heads (4 ACT ops -> 2 on the boundary-critical chain)
                o_ps = psO.tile([128, 1024], F32, tag="o",
                                name=f"ops{b}_{qb}")
                nch = 4 * qb + 4
                pending = None  # (p_t, off, j) awaiting AV matmuls

                def av(p_t, off, j):
                    for h in range(2):
                        lo = off if h == 0 else 512
                        nc.tensor.matmul(
                            o_ps[:, 512 * h + off: 512 * h + 512],
                            v_sb[b][:, j * 256 + h * 128: j * 256 + h * 128 + 128],
                            p_t[:, lo: lo + 512 - off],
                            start=(j == 0), stop=(j == nch - 1))

                for j in range(nch):
                    r = j - 4 * qb
                    off = 128 * r if r >= 0 else 0
                    s_ps = psA.tile([128, 1024], F32, tag="sA",
                                    name=f"s{b}_{qb}_{j}")
                    p_t = ppool.tile([128, 1024], BF16, tag="pt",
                                     name=f"p{b}_{qb}_{j}")
                    # h0's valid q-range lands at [off:512], h1's at
                    # [512:1024-off]: adjacent, so one exp covers both heads
                    for h in range(2):
                        lo = off if h == 0 else 512
                        nc.tensor.matmul(
                            s_ps[:, lo: lo + 512 - off],
                            kt[b][64 * h: 64 * h + 64, bass.ts(j, 128)],
                            qt[b][64 * h: 64 * h + 64,
                                  qb * 512 + off: qb * 512 + 512],
                            start=True, stop=True, tile_position=(64 * h, 0))
                    nc.scalar.activation(p_t[:, off: 1024 - off],
                                         s_ps[:, off: 1024 - off],
                                         EXPF, scale=0.125)
                    if r >= 0:
                        for h in range(2):
                            lo = off if h == 0 else 512
                            nc.vector.tensor_mul(
                                p_t[:, lo: lo + 128],
                                p_t[:, lo: lo + 128],
                                tri_sb[:])
                    if pending is not None:
                        av(*pending)
                    pending = (p_t, off, j)
                    pull(2)
                av(*pending)
                # bound the out-projection backlog so the o_sb slot chain
                # below can't deadlock (opool bufs exceeds backlog + 1);
                # the backlog doubles as PE filler for the last batch,
                # which has no projection work left to interleave - but
                # taper it off through that batch so nothing piles into a
                # serial drain after the last attention block
                # out-projection backlog: batch 0 drains promptly; batches
                # 1-2 defer (reserve PE work for the filler-poor tail);
                # batch 3 spends the reserve across its blocks, ending dry
                if b == 0:
                    limit = 5
                elif b < 3:
                    limit = 99
                else:
                    limit = max(0, 6 - 2 * qb)
                while len(fill_op) > limit:
                    pull_from(fill_op, 10 ** 9)

                # normalize: O[dv, q] / denom[q] (denom replicated on 64:128)
                o_sb = opool.tile([128, 512], BF16, tag="ot",
                                  name=f"o{b}_{qb}")
                lg = rpool.tile([64, 1024], F32, tag="lg",
                                name=f"lg{b}_{qb}")
                rec = rpool.tile([64, 1024], F32, tag="rc",
                                 name=f"r{b}_{qb}")
                _act_recip(nc, rec[:], o_ps[64:128, :], lg[:])
                for h in range(2):
                    nc.vector.tensor_mul(
                        o_sb[64 * h: 64 * h + 64, :],
                        o_ps[0:64, 512 * h: 512 * h + 512],
                        rec[:, 512 * h: 512 * h + 512])
                fill_op.append(outproj_gen(b, qb, o_sb))

            # ---- emission ----
            fill_proj.append((0, proj_gen(0)))
            force_proj(0, TPB)
            for b in range(B):
                if b + 1 < B:
                    # reserve the last batch's later t-blocks as filler for
                    # its own attention; earlier batches emit fully as filler
                    # of their predecessor
                    proj_cap[b + 1] = 1 if b + 1 == B - 1 else TPB
                    fill_proj.append((b + 1, proj_gen(b + 1)))
                if b == B - 1:
                    proj_cap[b] = TPB
                for qb in range(NQB):
                    # attention of block qb reads K/V only up to t-block qb
                    force_proj(b, min(qb + 1, TPB))
                    attn(b, qb)
            pull_from(fill_op, 10 ** 9)

    nc.compile()
    return nc


def _prep_inputs(x, W_qkv, b_qkv, W_out):
    bf = ml_dtypes.bfloat16
    flat = np.ascontiguousarray(x.reshape(BT, D))
    xT = np.ascontiguousarray(flat.T).astype(bf)
    tri = np.triu(np.ones((128, 128), np.float32)).astype(bf)
    ident = np.eye(128, dtype=np.float32).astype(bf)
    in_maps = []
    for c in range(N_CORES):
        sl = slice(128 * c, 128 * c + 128)
        in_maps.append({
            "xT": xT,
            "wq": np.ascontiguousarray(W_qkv[:, 0 * D:1 * D][:, sl]).astype(bf),
            "wk": np.ascontiguousarray(W_qkv[:, 1 * D:2 * D][:, sl]).astype(bf),
            "wv": np.ascontiguousarray(W_qkv[:, 2 * D:3 * D][:, sl]).astype(bf),
            "wo": np.ascontiguousarray(W_out[sl, :]).astype(bf),
            "bias": np.ascontiguousarray(np.stack(
                [b_qkv[0 * D:1 * D][sl], b_qkv[1 * D:2 * D][sl],
                 b_qkv[2 * D:3 * D][sl]], axis=1)).astype(np.float32),
            "tri": tri,
            "ident": ident,
        })
    return in_maps


def kernel(x, W_qkv, b_qkv, W_out, b_out):
    global _CACHED_NC, LAST_RESULTS
    x = np.asarray(x, np.float32)
    W_qkv = np.asarray(W_qkv, np.float32)
    b_qkv = np.asarray(b_qkv, np.float32)
    W_out = np.asarray(W_out, np.float32)
    b_out = np.asarray(b_out, np.float32)

    if _CACHED_NC is None:
        _CACHED_NC = _build()
    in_maps = _prep_inputs(x, W_qkv, b_qkv, W_out)
    res = run_bass_kernel_spmd(
        _CACHED_NC, in_maps, core_ids=list(range(N_CORES)),
        trace=bool(int(os.environ.get("ATTN_TRACE", "0"))))
    LAST_RESULTS = res
    acc = np.zeros((BT, D), np.float64)
    for r in res.results:
        acc += r["out"].astype(np.float64)
    out = (acc + b_out.astype(np.float64)).astype(np.float32)
    return out.reshape(B, T, D)



# revision 3
# speedup vs baseline: 1.0521x; 1.0521x over previous
"""Causal self-attention (B=4, T=2048, D=1024, H=16) on 8 TRN2 NeuronCores.

Sharding: batch x head-group. Core c owns batch c//2 and heads
[8*(c%2), 8*(c%2)+8). Each core projects its batch's tokens through its
512-column slice of W_qkv (column-parallel over heads), runs causal
attention for its 8 heads, and contracts its 512 rows of W_out into a
[2048, 1024] fp32 partial; the host adds the two partials per batch and
b_out. Per-core DMA is ~13MB (vs 48MB for pure head-TP) and the
out-projection reduction over this core's heads happens in PSUM.

Per-core kernel layout (all matmuls bf16 with fp32 PSUM accumulation):
  - x is pre-transposed on the host to xT [D, T].
  - Q^T/K^T [dh*2, t] per head-pair via W-stationary matmuls (contraction
    D on partitions, xT moving).
  - V is produced DIRECTLY as [t, dv] via x-stationary matmuls (xT chunk
    stationary, W_v moving) - no PE transposes, no assembly copies; one
    strided DVE copy scatters PSUM [128t, 512dv] into the per-head-pair
    [V_h | ones] slots around a pre-memset ones background.
  - Scores are computed transposed, S^T [keys, q], two heads packed into
    one PSUM tile via row-group tiling (contraction is 64 per head).
  - Softmax skips the max subtraction (scores are O(1) by construction),
    so exp comes straight off PSUM via ScalarE.
  - The AV stationary is [V_h | ones*64] (128 cols), so partitions
    64:128 of the O accumulator hold the softmax denominator replicated
    64x; the reciprocal runs on DVE (reciprocal_approx_fast, ~18 bits),
    keeping ScalarE exclusively for the exp stream (its critical path).
  - Causality: key-chunk matmuls on the diagonal are narrowed to the
    valid query range; the 128x128 boundary subtile is masked with a
    triangular constant after exp.
  - Out-projection accumulates over the 4 head-pairs in PSUM
    (start/stop flags), one [128, 512] bank at a time.
  - The attention inner loop is ACT(exp)-bound, so independent PE work
    (next t-block's projections, previous block's out-projection) is
    interleaved into it via filler generators to keep the TensorEngine
    dense and the HAM clock-gate warm.
"""
import os
import numpy as np
import ml_dtypes

import concourse.bass as bass
import concourse.tile as tile
from concourse import bacc, mybir
from concourse.bass_utils import run_bass_kernel_spmd

N_CORES = 8
B, T, D = 4, 2048, 1024
H, DH = 16, 64
HPC = 8                      # heads per core
NHP = 4                      # head pairs per core
TPB = T // 512               # 4 t-blocks
NQB = T // 512               # 4 query blocks
NKC = T // 128               # 16 key chunks

F32 = mybir.dt.float32
BF16 = mybir.dt.bfloat16
EXPF = mybir.ActivationFunctionType.Exp

_CACHED_NC = None
LAST_RESULTS = None  # test harness reads exec_time from here
DEBUG = bool(int(os.environ.get("ATTN_DEBUG", "0")))


def _build():
    nc = bacc.Bacc("TRN2", target_bir_lowering=False, debug=False,
                   num_devices=N_CORES)
    d_xT = nc.dram_tensor("xT", [D, T], BF16, kind="ExternalInput").ap()
    # wq/wk: stationary layout [p, cc(4), C(8), m(128)]
    d_wq = nc.dram_tensor("wq", [128, 4096], BF16, kind="ExternalInput").ap()
    d_wk = nc.dram_tensor("wk", [128, 4096], BF16, kind="ExternalInput").ap()
    # wv: moving layout [p, C(8), n(512)]
    d_wv = nc.dram_tensor("wv", [128, 4096], BF16, kind="ExternalInput").ap()
    # wo: moving layout [p, hp(4), n(1024)]
    d_wo = nc.dram_tensor("wo", [128, 4096], BF16, kind="ExternalInput").ap()
    d_bq = nc.dram_tensor("bq", [128, 4], F32, kind="ExternalInput").ap()
    d_bk = nc.dram_tensor("bk", [128, 4], F32, kind="ExternalInput").ap()
    # v-bias replicated across partitions, in PSUM-dv order
    d_bv = nc.dram_tensor("bv", [128, 512], F32, kind="ExternalInput").ap()
    d_tri = nc.dram_tensor("tri", [128, 128], BF16, kind="ExternalInput").ap()
    d_out = nc.dram_tensor("out", [T, D], F32, kind="ExternalOutput").ap()
    if DEBUG:
        d_dbg = {
            "dbg_qt0": nc.dram_tensor("dbg_qt0", [128, T], BF16,
                                      kind="ExternalOutput").ap(),
            "dbg_kt0": nc.dram_tensor("dbg_kt0", [128, T], BF16,
                                      kind="ExternalOutput").ap(),
            "dbg_v": nc.dram_tensor("dbg_v", [128, 4 * NKC * 256], BF16,
                                    kind="ExternalOutput").ap(),
            "dbg_osb": nc.dram_tensor("dbg_osb", [128, 512], BF16,
                                      kind="ExternalOutput").ap(),
            "dbg_rec": nc.dram_tensor("dbg_rec", [64, 1024], F32,
                                      kind="ExternalOutput").ap(),
            "dbg_pt": nc.dram_tensor("dbg_pt", [128, 1024], BF16,
                                     kind="ExternalOutput").ap(),
        }

    with tile.TileContext(nc) as tc:
        with tc.tile_pool(name="consts", bufs=1) as consts, \
             tc.tile_pool(name="big", bufs=1) as big, \
             tc.tile_pool(name="xt", bufs=2) as xpool, \
             tc.tile_pool(name="pt", bufs=6) as ppool, \
             tc.tile_pool(name="ot", bufs=9) as opool, \
             tc.tile_pool(name="rc", bufs=2) as rpool, \
             tc.tile_pool(name="outp", bufs=4) as outp, \
             tc.tile_pool(name="psS", bufs=2, space="PSUM") as psS, \
             tc.tile_pool(name="psO", bufs=1, space="PSUM") as psO, \
             tc.tile_pool(name="psX", bufs=2, space="PSUM") as psX:

            # ---- constants ----
            wq_sb = consts.tile([128, 4096], BF16, tag="wq")
            bq_sb = consts.tile([128, 4], F32, tag="bq")
            wk_sb = consts.tile([128, 4096], BF16, tag="wk")
            bk_sb = consts.tile([128, 4], F32, tag="bk")
            wv_sb = consts.tile([128, 4096], BF16, tag="wv")
            bv_sb = consts.tile([128, 512], F32, tag="bv")
            tri_sb = consts.tile([128, 128], BF16, tag="tri")
            wo_sb = consts.tile([128, 4096], BF16, tag="wo")
            dummy = consts.tile([128, 1], BF16, tag="dumm")
            # chunked so the first Q matmuls wait only on the first chunk
            for cc in range(4):
                nc.sync.dma_start(wq_sb[:, bass.ts(cc, 1024)],
                                  d_wq[:, bass.ts(cc, 1024)])
            nc.sync.dma_start(bq_sb[:], d_bq[:])
            # touch Exp now so the ACT table load overlaps the projections
            nc.scalar.activation(dummy[:], bq_sb[:, 0:1], EXPF)
            for cc in range(4):
                nc.sync.dma_start(wk_sb[:, bass.ts(cc, 1024)],
                                  d_wk[:, bass.ts(cc, 1024)])
            nc.sync.dma_start(bk_sb[:], d_bk[:])
            nc.sync.dma_start(wv_sb[:], d_wv[:])
            nc.sync.dma_start(bv_sb[:], d_bv[:])
            nc.sync.dma_start(tri_sb[:], d_tri[:])
            nc.sync.dma_start(wo_sb[:], d_wo[:])

            # ---- persistent tensors ----
            # qt/kt[hp]: [2 heads * 64 dh, T] transposed projections
            qt = [big.tile([128, T], BF16, tag=f"qt{p}", name=f"qt{p}")
                  for p in range(NHP)]
            kt = [big.tile([128, T], BF16, tag=f"kt{p}", name=f"kt{p}")
                  for p in range(NHP)]
            # v_all: per hp (stride 4096), per key chunk j (stride 256):
            # [V_h0 (64) | ones (64) | V_h1 (64) | ones (64)]
            v_all = big.tile([128, NHP * NKC * 256], BF16, tag="v")
            # DVE memset: keeps the gpsimd queue free for the x DMA issues
            nc.vector.memset(v_all[:], 1.0)

            proj_emitted = [0]   # t-blocks fully emitted
            fill_proj = []       # projection generators (one per t-block)
            fill_op = []         # out-projection generators

            def qk_step(x_t, tb, w_sb, b_sb, dest, cc):
                ps = psX.tile([128, 512], F32, tag="aux",
                              name=f"pj{tb}_{cc}")
                for c in range(8):
                    nc.tensor.matmul(
                        ps[:],
                        w_sb[:, cc * 1024 + c * 128:
                             cc * 1024 + c * 128 + 128],
                        x_t[:, bass.ts(c, 512)],
                        start=(c == 0), stop=(c == 7))
                    yield
                nc.vector.tensor_scalar_add(
                    dest[cc][:, bass.ts(tb, 512)], ps[:],
                    b_sb[:, cc:cc + 1])
                yield

            def v_step(x_t, tb, tsub):
                # V: x-stationary, lands as [t, dv] directly
                ps = psX.tile([128, 512], F32, tag="aux",
                              name=f"pv{tb}_{tsub}")
                for c in range(8):
                    nc.tensor.matmul(
                        ps[:],
                        x_t[:, c * 512 + tsub * 128:
                            c * 512 + tsub * 128 + 128],
                        wv_sb[:, bass.ts(c, 512)],
                        start=(c == 0), stop=(c == 7))
                    yield
                j = tb * 4 + tsub
                # scatter [128, (hp,h,dv)] into the [V|1|V|1] slots
                d0 = v_all[:, j * 256: j * 256 + 64]
                dst = bass.AP(d0.tensor, d0.offset,
                              [d0.ap[0], [4096, 4], [128, 2], [1, 64]])
                nc.vector.tensor_add(dst, ps[:, 0:512], bv_sb[:, 0:512])
                yield

            def proj_gen(tb):
                """Q/K/V projections of one t-block in small PE steps.
                Emission order Q0,K0,V*,Q1,K1,... lets attention on head
                pair 0 start while later head pairs still project."""
                x_t = xpool.tile([128, 8 * 512], BF16, tag="xt",
                                 name=f"x{tb}")
                for c in range(8):
                    # gpsimd queue: runs parallel to the const DMAs (sync)
                    nc.gpsimd.dma_start(
                        x_t[:, bass.ts(c, 512)],
                        d_xT[c * 128: c * 128 + 128, bass.ts(tb, 512)])
                for cc in range(4):
                    yield from qk_step(x_t, tb, wq_sb, bq_sb, qt, cc)
                    yield from qk_step(x_t, tb, wk_sb, bk_sb, kt, cc)
                    if cc == 0:
                        for tsub in range(4):
                            yield from v_step(x_t, tb, tsub)
                proj_emitted[0] = tb + 1

            def outproj_gen(qb, osbs):
                """out[q, n] += sum_hp o_sb[hp]^T @ wo[hp], per q-chunk."""
                for qc in range(4):
                    ops = [psX.tile([128, 512], F32, tag="aux",
                                    name=f"op{qb}_{qc}_{nh}")
                           for nh in range(2)]
                    for hp in range(NHP):
                        for nh in range(2):
                            nc.tensor.matmul(
                                ops[nh][:],
                                osbs[hp][:, bass.ts(qc, 128)],
                                wo_sb[:, hp * 1024 + nh * 512:
                                      hp * 1024 + nh * 512 + 512],
                                start=(hp == 0), stop=(hp == 3))
                            yield
                    row = qb * 512 + qc * 128
                    for nh in range(2):
                        ob = outp.tile([128, 512], F32, tag="outp",
                                       name=f"ob{qb}_{qc}_{nh}")
                        nc.vector.tensor_copy(ob[:], ops[nh][:])
                        nc.sync.dma_start(
                            d_out[row:row + 128, bass.ts(nh, 512)], ob[:])
                        yield

            def pull_from(lst, n):
                for _ in range(n):
                    while lst:
                        try:
                            next(lst[0])
                            break
                        except StopIteration:
                            lst.pop(0)
                    else:
                        break

            def force_proj(upto):
                """Emit projections through t-block `upto` NOW."""
                while proj_emitted[0] < upto and fill_proj:
                    try:
                        next(fill_proj[0])
                    except StopIteration:
                        fill_proj.pop(0)

            def pull(n):
                pull_from(fill_op, 1)
                pull_from(fill_proj, n)

            def attn(hp, qb):
                """Attention for query block qb, head pair hp. Returns the
                normalized [128, 512] bf16 O^T tile."""
                o_ps = psO.tile([128, 1024], F32, tag="o",
                                name=f"ops{hp}_{qb}")
                nch = 4 * qb + 4
                pending = None

                def av(p_t, off, j):
                    for h in range(2):
                        lo = off if h == 0 else 512
                        nc.tensor.matmul(
                            o_ps[:, 512 * h + off: 512 * h + 512],
                            v_all[:, hp * 4096 + j * 256 + h * 128:
                                  hp * 4096 + j * 256 + h * 128 + 128],
                            p_t[:, lo: lo + 512 - off],
                            start=(j == 0), stop=(j == nch - 1))

                for j in range(nch):
                    r = j - 4 * qb
                    off = 128 * r if r >= 0 else 0
                    s_ps = psS.tile([128, 1024], F32, tag="s",
                                    name=f"s{hp}_{qb}_{j}")
                    p_t = ppool.tile([128, 1024], BF16, tag="pt",
                                     name=f"p{hp}_{qb}_{j}")
                    for h in range(2):
                        lo = off if h == 0 else 512
                        nc.tensor.matmul(
                            s_ps[:, lo: lo + 512 - off],
                            kt[hp][64 * h: 64 * h + 64, bass.ts(j, 128)],
                            qt[hp][64 * h: 64 * h + 64,
                                   qb * 512 + off: qb * 512 + 512],
                            start=True, stop=True, tile_position=(64 * h, 0))
                    nc.scalar.activation(p_t[:, off: 1024 - off],
                                         s_ps[:, off: 1024 - off],
                                         EXPF, scale=0.125)
                    if r >= 0:
                        for h in range(2):
                            lo = off if h == 0 else 512
                            nc.vector.tensor_mul(
                                p_t[:, lo: lo + 128],
                                p_t[:, lo: lo + 128],
                                tri_sb[:])
                    if DEBUG and hp == 0 and qb == 0 and j == 0:
                        nc.sync.dma_start(d_dbg["dbg_pt"][:], p_t[:])
                    if pending is not None:
                        av(*pending)
                    pending = (p_t, off, j)
                    # early query blocks have few chunks but a whole
                    # t-block of projections to interleave: pull harder
                    pull(max(2, 7 - 2 * qb))
                av(*pending)

                # normalize: O[dv, q] / denom[q] (denom replicated on 64:128)
                o_sb = opool.tile([128, 512], BF16, tag="ot",
                                  name=f"o{hp}_{qb}")
                den = rpool.tile([64, 1024], F32, tag="dn",
                                 name=f"d{hp}_{qb}")
                rec = rpool.tile([64, 1024], F32, tag="rc",
                                 name=f"r{hp}_{qb}")
                # custom-DVE ops misread PSUM on HW: stage denom in SBUF
                nc.vector.tensor_copy(den[:], o_ps[64:128, :])
                nc.vector.reciprocal_approx_fast(rec[:], den[:])
                for h in range(2):
                    nc.vector.tensor_mul(
                        o_sb[64 * h: 64 * h + 64, :],
                        o_ps[0:64, bass.ts(h, 512)],
                        rec[:, bass.ts(h, 512)])
                if DEBUG and hp == 0 and qb == 0:
                    nc.sync.dma_start(d_dbg["dbg_rec"][:], rec[:])
                    nc.sync.dma_start(d_dbg["dbg_osb"][:], o_sb[:])
                return o_sb

            # ---- emission ----
            fill_proj.append(proj_gen(0))
            for qb in range(NQB):
                force_proj(qb + 1)
                if qb + 1 < TPB:
                    fill_proj.append(proj_gen(qb + 1))
                osbs = []
                for hp in range(NHP):
                    osbs.append(attn(hp, qb))
                fill_op.append(outproj_gen(qb, osbs))
            pull_from(fill_proj, 10 ** 9)
            pull_from(fill_op, 10 ** 9)
            if DEBUG:
                nc.sync.dma_start(d_dbg["dbg_qt0"][:], qt[0][:])
                nc.sync.dma_start(d_dbg["dbg_kt0"][:], kt[0][:])
                nc.sync.dma_start(d_dbg["dbg_v"][:], v_all[:])

    nc.compile()
    return nc


def _prep_inputs(x, W_qkv, b_qkv, W_out):
    bf = ml_dtypes.bfloat16
    tri = np.triu(np.ones((128, 128), np.float32)).astype(bf)
    in_maps = []
    for c in range(N_CORES):
        b, hg = c // 2, c % 2
        sl = slice(hg * 512, hg * 512 + 512)
        xT = np.ascontiguousarray(x[b].T).astype(bf)          # [D, T]
        Wq = W_qkv[:, 0 * D:1 * D][:, sl]                     # [D, 512]
        Wk = W_qkv[:, 1 * D:2 * D][:, sl]
        Wv = W_qkv[:, 2 * D:3 * D][:, sl]
        Wo = W_out[sl, :]                                     # [512, D]
        # [p, cc, C, m]: element [C*128+p, cc*128+m]
        wq = np.ascontiguousarray(
            Wq.reshape(8, 128, 4, 128).transpose(1, 2, 0, 3)
        ).reshape(128, 4096).astype(bf)
        wk = np.ascontiguousarray(
            Wk.reshape(8, 128, 4, 128).transpose(1, 2, 0, 3)
        ).reshape(128, 4096).astype(bf)
        # [p, C, n]: element [C*128+p, n]
        wv = np.ascontiguousarray(
            Wv.reshape(8, 128, 512).transpose(1, 0, 2)
        ).reshape(128, 4096).astype(bf)
        # [p, hp, n]: element [hp*128+p, n]
        wo = np.ascontiguousarray(
            Wo.reshape(4, 128, 1024).transpose(1, 0, 2)
        ).reshape(128, 4096).astype(bf)
        bq = np.ascontiguousarray(
            b_qkv[0 * D:1 * D][sl].reshape(4, 128).T).astype(np.float32)
        bk = np.ascontiguousarray(
            b_qkv[1 * D:2 * D][sl].reshape(4, 128).T).astype(np.float32)
        bv = np.broadcast_to(
            b_qkv[2 * D:3 * D][sl][None, :], (128, 512))
        bv = np.ascontiguousarray(bv).astype(np.float32)
        in_maps.append({
            "xT": xT, "wq": wq, "wk": wk, "wv": wv, "wo": wo,
            "bq": bq, "bk": bk, "bv": bv, "tri": tri,
        })
    return in_maps


def kernel(x, W_qkv, b_qkv, W_out, b_out):
    global _CACHED_NC, LAST_RESULTS
    x = np.asarray(x, np.float32)
    W_qkv = np.asarray(W_qkv, np.float32)
    b_qkv = np.asarray(b_qkv, np.float32)
    W_out = np.asarray(W_out, np.float32)
    b_out = np.asarray(b_out, np.float32)

    if _CACHED_NC is None:
        _CACHED_NC = _build()
    in_maps = _prep_inputs(x, W_qkv, b_qkv, W_out)
    res = run_bass_kernel_spmd(
        _CACHED_NC, in_maps, core_ids=list(range(N_CORES)),
        trace=bool(int(os.environ.get("ATTN_TRACE", "0"))))
    LAST_RESULTS = res
    out = np.zeros((B, T, D), np.float32)
    bo = b_out.astype(np.float64)
    for b in range(B):
        acc = (res.results[2 * b]["out"].astype(np.float64)
               + res.results[2 * b + 1]["out"].astype(np.float64) + bo)
        out[b] = acc.astype(np.float32)
    return out


# revision 4
# speedup vs baseline: 1.0554x; 1.0031x over previous
"""Causal self-attention (B=4, T=2048, D=1024, H=16) on 8 TRN2 NeuronCores.

Sharding: batch x head-group. Core c owns batch c//2 and heads
[8*(c%2), 8*(c%2)+8). Each core projects its batch's tokens through its
512-column slice of W_qkv (column-parallel over heads), runs causal
attention for its 8 heads, and contracts its 512 rows of W_out into a
[2048, 1024] fp32 partial; the host adds the two partials per batch and
b_out. Per-core DMA is ~13MB (vs 48MB for pure head-TP) and the
out-projection reduction over this core's heads happens in PSUM.

Per-core kernel layout (all matmuls bf16 with fp32 PSUM accumulation):
  - x is pre-transposed on the host to xT [D, T].
  - Q^T/K^T [dh*2, t] per head-pair via W-stationary matmuls (contraction
    D on partitions, xT moving).
  - V is produced DIRECTLY as [t, dv] via x-stationary matmuls (xT chunk
    stationary, W_v moving) - no PE transposes, no assembly copies; one
    strided DVE copy scatters PSUM [128t, 512dv] into the per-head-pair
    [V_h | ones] slots around a pre-memset ones background.
  - Scores are computed transposed, S^T [keys, q], two heads packed into
    one PSUM tile via row-group tiling (contraction is 64 per head).
  - Softmax skips the max subtraction (scores are O(1) by construction),
    so exp comes straight off PSUM via ScalarE.
  - The AV stationary is [V_h | ones*64] (128 cols), so partitions
    64:128 of the O accumulator hold the softmax denominator replicated
    64x; the reciprocal runs on DVE (reciprocal_approx_fast, ~18 bits),
    keeping ScalarE exclusively for the exp stream (its critical path).
  - Causality: key-chunk matmuls on the diagonal are narrowed to the
    valid query range; the 128x128 boundary subtile is masked with a
    triangular constant after exp.
  - Out-projection accumulates over the 4 head-pairs in PSUM
    (start/stop flags), one [128, 512] bank at a time.
  - The attention inner loop is ACT(exp)-bound, so independent PE work
    (next t-block's projections, previous block's out-projection) is
    interleaved into it via filler generators to keep the TensorEngine
    dense and the HAM clock-gate warm.
"""
import os
import numpy as np
import ml_dtypes

import concourse.bass as bass
import concourse.tile as tile
from concourse import bacc, mybir
from concourse.bass_utils import run_bass_kernel_spmd

N_CORES = 8
B, T, D = 4, 2048, 1024
H, DH = 16, 64
HPC = 8                      # heads per core
NHP = 4                      # head pairs per core
TPB = T // 512               # 4 t-blocks
NQB = T // 512               # 4 query blocks
NKC = T // 128               # 16 key chunks

F32 = mybir.dt.float32
BF16 = mybir.dt.bfloat16
EXPF = mybir.ActivationFunctionType.Exp

_CACHED_NC = None
LAST_RESULTS = None  # test harness reads exec_time from here
DEBUG = bool(int(os.environ.get("ATTN_DEBUG", "0")))


def _build():
    nc = bacc.Bacc("TRN2", target_bir_lowering=False, debug=False,
                   num_devices=N_CORES)
    d_xT = nc.dram_tensor("xT", [D, T], BF16, kind="ExternalInput").ap()
    # wq/wk: stationary layout [p, cc(4), C(8), m(128)]
    d_wq = nc.dram_tensor("wq", [128, 4096], BF16, kind="ExternalInput").ap()
    d_wk = nc.dram_tensor("wk", [128, 4096], BF16, kind="ExternalInput").ap()
    # wv: moving layout [p, C(8), n(512)]
    d_wv = nc.dram_tensor("wv", [128, 4096], BF16, kind="ExternalInput").ap()
    # wo: moving layout [p, hp(4), n(1024)]
    d_wo = nc.dram_tensor("wo", [128, 4096], BF16, kind="ExternalInput").ap()
    d_bq = nc.dram_tensor("bq", [128, 4], F32, kind="ExternalInput").ap()
    d_bk = nc.dram_tensor("bk", [128, 4], F32, kind="ExternalInput").ap()
    # v-bias replicated across partitions, in PSUM-dv order
    d_bv = nc.dram_tensor("bv", [128, 512], F32, kind="ExternalInput").ap()
    d_tri = nc.dram_tensor("tri", [128, 128], BF16, kind="ExternalInput").ap()
    d_out = nc.dram_tensor("out", [T, D], F32, kind="ExternalOutput").ap()
    if DEBUG:
        d_dbg = {
            "dbg_qt0": nc.dram_tensor("dbg_qt0", [128, T], BF16,
                                      kind="ExternalOutput").ap(),
            "dbg_kt0": nc.dram_tensor("dbg_kt0", [128, T], BF16,
                                      kind="ExternalOutput").ap(),
            "dbg_v": nc.dram_tensor("dbg_v", [128, 4 * NKC * 256], BF16,
                                    kind="ExternalOutput").ap(),
            "dbg_osb": nc.dram_tensor("dbg_osb", [128, 512], BF16,
                                      kind="ExternalOutput").ap(),
            "dbg_rec": nc.dram_tensor("dbg_rec", [64, 1024], F32,
                                      kind="ExternalOutput").ap(),
            "dbg_pt": nc.dram_tensor("dbg_pt", [128, 1024], BF16,
                                     kind="ExternalOutput").ap(),
        }

    with tile.TileContext(nc) as tc:
        with tc.tile_pool(name="consts", bufs=1) as consts, \
             tc.tile_pool(name="big", bufs=1) as big, \
             tc.tile_pool(name="xt", bufs=2) as xpool, \
             tc.tile_pool(name="pt", bufs=6) as ppool, \
             tc.tile_pool(name="ot", bufs=9) as opool, \
             tc.tile_pool(name="rc", bufs=2) as rpool, \
             tc.tile_pool(name="outp", bufs=4) as outp, \
             tc.tile_pool(name="psS", bufs=2, space="PSUM") as psS, \
             tc.tile_pool(name="psO", bufs=1, space="PSUM") as psO, \
             tc.tile_pool(name="psX", bufs=2, space="PSUM") as psX:

            # ---- constants ----
            wq_sb = consts.tile([128, 4096], BF16, tag="wq")
            bq_sb = consts.tile([128, 4], F32, tag="bq")
            wk_sb = consts.tile([128, 4096], BF16, tag="wk")
            bk_sb = consts.tile([128, 4], F32, tag="bk")
            wv_sb = consts.tile([128, 4096], BF16, tag="wv")
            bv_sb = consts.tile([128, 512], F32, tag="bv")
            tri_sb = consts.tile([128, 128], BF16, tag="tri")
            wo_sb = consts.tile([128, 4096], BF16, tag="wo")
            dummy = consts.tile([128, 1], BF16, tag="dumm")
            # chunked so the first Q matmuls wait only on the first chunk
            for cc in range(4):
                nc.sync.dma_start(wq_sb[:, bass.ts(cc, 1024)],
                                  d_wq[:, bass.ts(cc, 1024)])
            nc.sync.dma_start(bq_sb[:], d_bq[:])
            # touch Exp now so the ACT table load overlaps the projections
            nc.scalar.activation(dummy[:], bq_sb[:, 0:1], EXPF)
            for cc in range(4):
                nc.sync.dma_start(wk_sb[:, bass.ts(cc, 1024)],
                                  d_wk[:, bass.ts(cc, 1024)])
            nc.sync.dma_start(bk_sb[:], d_bk[:])
            nc.sync.dma_start(wv_sb[:], d_wv[:])
            nc.sync.dma_start(bv_sb[:], d_bv[:])
            nc.sync.dma_start(tri_sb[:], d_tri[:])
            nc.sync.dma_start(wo_sb[:], d_wo[:])

            # ---- persistent tensors ----
            # qt/kt[hp]: [2 heads * 64 dh, T] transposed projections
            qt = [big.tile([128, T], BF16, tag=f"qt{p}", name=f"qt{p}")
                  for p in range(NHP)]
            kt = [big.tile([128, T], BF16, tag=f"kt{p}", name=f"kt{p}")
                  for p in range(NHP)]
            # v_all: per hp (stride 4096), per key chunk j (stride 256):
            # [V_h0 (64) | ones (64) | V_h1 (64) | ones (64)]
            v_all = big.tile([128, NHP * NKC * 256], BF16, tag="v")

            proj_emitted = [0]   # t-blocks fully emitted
            proj_state = {}      # tb -> {"qk": [bool]*4, "v": bool}
            fill_proj = []       # projection generators (one per t-block)
            fill_op = []         # out-projection generators

            def qk_step(x_t, tb, w_sb, b_sb, dest, cc):
                ps = psX.tile([128, 512], F32, tag="aux",
                              name=f"pj{tb}_{cc}")
                for c in range(8):
                    nc.tensor.matmul(
                        ps[:],
                        w_sb[:, cc * 1024 + c * 128:
                             cc * 1024 + c * 128 + 128],
                        x_t[:, bass.ts(c, 512)],
                        start=(c == 0), stop=(c == 7))
                    yield
                nc.vector.tensor_scalar_add(
                    dest[cc][:, bass.ts(tb, 512)], ps[:],
                    b_sb[:, cc:cc + 1])
                yield

            def v_step(x_t, tb, tsub):
                # V: x-stationary, lands as [t, dv] directly
                ps = psX.tile([128, 512], F32, tag="aux",
                              name=f"pv{tb}_{tsub}")
                for c in range(8):
                    nc.tensor.matmul(
                        ps[:],
                        x_t[:, c * 512 + tsub * 128:
                            c * 512 + tsub * 128 + 128],
                        wv_sb[:, bass.ts(c, 512)],
                        start=(c == 0), stop=(c == 7))
                    yield
                j = tb * 4 + tsub
                # scatter [128, (hp,h,dv)] into the [V|1|V|1] slots
                d0 = v_all[:, j * 256: j * 256 + 64]
                dst = bass.AP(d0.tensor, d0.offset,
                              [d0.ap[0], [4096, 4], [128, 2], [1, 64]])
                nc.vector.tensor_add(dst, ps[:, 0:512], bv_sb[:, 0:512])
                yield

            def proj_gen(tb):
                """Q/K/V projections of one t-block in small PE steps.
                Emission order Q0,K0,V*,Q1,K1,... + progress markers let
                attention on head pair hp start as soon as ITS Q/K chunk
                is in flight, with the rest of the t-block as filler."""
                st = proj_state[tb] = {"qk": [False] * 4, "v": False}

                def gen():
                    x_t = xpool.tile([128, 8 * 512], BF16, tag="xt",
                                     name=f"x{tb}")
                    for c in range(8):
                        # gpsimd queue: parallel to the const DMAs (sync)
                        nc.gpsimd.dma_start(
                            x_t[:, bass.ts(c, 512)],
                            d_xT[c * 128: c * 128 + 128, bass.ts(tb, 512)])
                    for cc in range(4):
                        yield from qk_step(x_t, tb, wq_sb, bq_sb, qt, cc)
                        yield from qk_step(x_t, tb, wk_sb, bk_sb, kt, cc)
                        st["qk"][cc] = True
                        if cc == 0:
                            if tb == 0:
                                # ones background; gpsimd so the DVE is
                                # free for the first qt/kt bias copies
                                nc.gpsimd.memset(v_all[:], 1.0)
                            for tsub in range(4):
                                yield from v_step(x_t, tb, tsub)
                            st["v"] = True
                    proj_emitted[0] = tb + 1
                return gen()

            def outproj_gen(qb, osbs):
                """out[q, n] += sum_hp o_sb[hp]^T @ wo[hp], per q-chunk."""
                for qc in range(4):
                    ops = [psX.tile([128, 512], F32, tag="aux",
                                    name=f"op{qb}_{qc}_{nh}")
                           for nh in range(2)]
                    for hp in range(NHP):
                        for nh in range(2):
                            nc.tensor.matmul(
                                ops[nh][:],
                                osbs[hp][:, bass.ts(qc, 128)],
                                wo_sb[:, hp * 1024 + nh * 512:
                                      hp * 1024 + nh * 512 + 512],
                                start=(hp == 0), stop=(hp == 3))
                            yield
                    row = qb * 512 + qc * 128
                    for nh in range(2):
                        ob = outp.tile([128, 512], F32, tag="outp",
                                       name=f"ob{qb}_{qc}_{nh}")
                        nc.vector.tensor_copy(ob[:], ops[nh][:])
                        nc.sync.dma_start(
                            d_out[row:row + 128, bass.ts(nh, 512)], ob[:])
                        yield

            def pull_from(lst, n):
                for _ in range(n):
                    while lst:
                        try:
                            next(lst[0])
                            break
                        except StopIteration:
                            lst.pop(0)
                    else:
                        break

            def force_until(pred):
                """Emit projection steps until pred() holds."""
                while not pred() and fill_proj:
                    try:
                        next(fill_proj[0])
                    except StopIteration:
                        fill_proj.pop(0)

            def pull(n):
                pull_from(fill_op, 2)
                pull_from(fill_proj, n)

            def attn(hp, qb):
                """Attention for query block qb, head pair hp. Returns the
                normalized [128, 512] bf16 O^T tile."""
                # gate only on THIS head pair's Q/K chunk of t-block qb
                force_until(lambda: proj_state[qb]["qk"][hp])
                o_ps = psO.tile([128, 1024], F32, tag="o",
                                name=f"ops{hp}_{qb}")
                nch = 4 * qb + 4
                pending = None

                def av(p_t, off, j):
                    for h in range(2):
                        lo = off if h == 0 else 512
                        nc.tensor.matmul(
                            o_ps[:, 512 * h + off: 512 * h + 512],
                            v_all[:, hp * 4096 + j * 256 + h * 128:
                                  hp * 4096 + j * 256 + h * 128 + 128],
                            p_t[:, lo: lo + 512 - off],
                            start=(j == 0), stop=(j == nch - 1))

                for j in range(nch):
                    r = j - 4 * qb
                    off = 128 * r if r >= 0 else 0
                    s_ps = psS.tile([128, 1024], F32, tag="s",
                                    name=f"s{hp}_{qb}_{j}")
                    p_t = ppool.tile([128, 1024], BF16, tag="pt",
                                     name=f"p{hp}_{qb}_{j}")
                    for h in range(2):
                        lo = off if h == 0 else 512
                        nc.tensor.matmul(
                            s_ps[:, lo: lo + 512 - off],
                            kt[hp][64 * h: 64 * h + 64, bass.ts(j, 128)],
                            qt[hp][64 * h: 64 * h + 64,
                                   qb * 512 + off: qb * 512 + 512],
                            start=True, stop=True, tile_position=(64 * h, 0))
                    nc.scalar.activation(p_t[:, off: 1024 - off],
                                         s_ps[:, off: 1024 - off],
                                         EXPF, scale=0.125)
                    if r >= 0:
                        for h in range(2):
                            lo = off if h == 0 else 512
                            nc.vector.tensor_mul(
                                p_t[:, lo: lo + 128],
                                p_t[:, lo: lo + 128],
                                tri_sb[:])
                    if DEBUG and hp == 0 and qb == 0 and j == 0:
                        nc.sync.dma_start(d_dbg["dbg_pt"][:], p_t[:])
                    if pending is not None:
                        if pending[2] == 0:
                            # first AV reads the ones columns + V chunks
                            force_until(lambda: proj_state[qb]["v"])
                        av(*pending)
                    pending = (p_t, off, j)
                    # early query blocks have few chunks but a whole
                    # t-block of projections to interleave: pull harder
                    pull(max(2, 7 - 2 * qb))
                if pending[2] == 0:
                    force_until(lambda: proj_state[qb]["v"])
                av(*pending)

                # normalize: O[dv, q] / denom[q] (denom replicated on 64:128)
                o_sb = opool.tile([128, 512], BF16, tag="ot",
                                  name=f"o{hp}_{qb}")
                den = rpool.tile([64, 1024], F32, tag="dn",
                                 name=f"d{hp}_{qb}")
                rec = rpool.tile([64, 1024], F32, tag="rc",
                                 name=f"r{hp}_{qb}")
                # custom-DVE ops misread PSUM on HW: stage denom in SBUF
                nc.vector.tensor_copy(den[:], o_ps[64:128, :])
                nc.vector.reciprocal_approx_fast(rec[:], den[:])
                for h in range(2):
                    nc.vector.tensor_mul(
                        o_sb[64 * h: 64 * h + 64, :],
                        o_ps[0:64, bass.ts(h, 512)],
                        rec[:, bass.ts(h, 512)])
                if DEBUG and hp == 0 and qb == 0:
                    nc.sync.dma_start(d_dbg["dbg_rec"][:], rec[:])
                    nc.sync.dma_start(d_dbg["dbg_osb"][:], o_sb[:])
                return o_sb

            # ---- emission ----
            fill_proj.append(proj_gen(0))
            for qb in range(NQB):
                if qb + 1 < TPB:
                    fill_proj.append(proj_gen(qb + 1))
                osbs = []
                for hp in range(NHP):
                    osbs.append(attn(hp, qb))
                fill_op.append(outproj_gen(qb, osbs))
            pull_from(fill_proj, 10 ** 9)
            pull_from(fill_op, 10 ** 9)
            if DEBUG:
                nc.sync.dma_start(d_dbg["dbg_qt0"][:], qt[0][:])
                nc.sync.dma_start(d_dbg["dbg_kt0"][:], kt[0][:])
                nc.sync.dma_start(d_dbg["dbg_v"][:], v_all[:])

    nc.compile()
    return nc


def _prep_inputs(x, W_qkv, b_qkv, W_out):
    bf = ml_dtypes.bfloat16
    tri = np.triu(np.ones((128, 128), np.float32)).astype(bf)
    in_maps = []
    for c in range(N_CORES):
        b, hg = c // 2, c % 2
        sl = slice(hg * 512, hg * 512 + 512)
        xT = np.ascontiguousarray(x[b].T).astype(bf)          # [D, T]
        Wq = W_qkv[:, 0 * D:1 * D][:, sl]                     # [D, 512]
        Wk = W_qkv[:, 1 * D:2 * D][:, sl]
        Wv = W_qkv[:, 2 * D:3 * D][:, sl]
        Wo = W_out[sl, :]                                     # [512, D]
        # [p, cc, C, m]: element [C*128+p, cc*128+m]
        wq = np.ascontiguousarray(
            Wq.reshape(8, 128, 4, 128).transpose(1, 2, 0, 3)
        ).reshape(128, 4096).astype(bf)
        wk = np.ascontiguousarray(
            Wk.reshape(8, 128, 4, 128).transpose(1, 2, 0, 3)
        ).reshape(128, 4096).astype(bf)
        # [p, C, n]: element [C*128+p, n]
        wv = np.ascontiguousarray(
            Wv.reshape(8, 128, 512).transpose(1, 0, 2)
        ).reshape(128, 4096).astype(bf)
        # [p, hp, n]: element [hp*128+p, n]
        wo = np.ascontiguousarray(
            Wo.reshape(4, 128, 1024).transpose(1, 0, 2)
        ).reshape(128, 4096).astype(bf)
        bq = np.ascontiguousarray(
            b_qkv[0 * D:1 * D][sl].reshape(4, 128).T).astype(np.float32)
        bk = np.ascontiguousarray(
            b_qkv[1 * D:2 * D][sl].reshape(4, 128).T).astype(np.float32)
        bv = np.broadcast_to(
            b_qkv[2 * D:3 * D][sl][None, :], (128, 512))
        bv = np.ascontiguousarray(bv).astype(np.float32)
        in_maps.append({
            "xT": xT, "wq": wq, "wk": wk, "wv": wv, "wo": wo,
            "bq": bq, "bk": bk, "bv": bv, "tri": tri,
        })
    return in_maps


def kernel(x, W_qkv, b_qkv, W_out, b_out):
    global _CACHED_NC, LAST_RESULTS
    x = np.asarray(x, np.float32)
    W_qkv = np.asarray(W_qkv, np.float32)
    b_qkv = np.asarray(b_qkv, np.float32)
    W_out = np.asarray(W_out, np.float32)
    b_out = np.asarray(b_out, np.float32)

    if _CACHED_NC is None:
        _CACHED_NC = _build()
    in_maps = _prep_inputs(x, W_qkv, b_qkv, W_out)
    res = run_bass_kernel_spmd(
        _CACHED_NC, in_maps, core_ids=list(range(N_CORES)),
        trace=bool(int(os.environ.get("ATTN_TRACE", "0"))))
    LAST_RESULTS = res
    out = np.zeros((B, T, D), np.float32)
    bo = b_out.astype(np.float64)
    for b in range(B):
        acc = (res.results[2 * b]["out"].astype(np.float64)
               + res.results[2 * b + 1]["out"].astype(np.float64) + bo)
        out[b] = acc.astype(np.float32)
    return out
